# revision 8
# baseline (speedup 1.0000x reference)
"""Distributed Trainium2 kernel for a dense transformer block (v4).

Sharding: sequence-parallel over the 8 NeuronCores. The flattened
[B*S=4096, D=1024] token stream is split into 8 contiguous shards of 512
tokens (cores 0-3 hold batch 0, cores 4-7 hold batch 1). Weights are
replicated. Collectives: an AllGather of K^T right after the K GEMM and
a second AllGather of V right after the V GEMM, both within the 4-core
batch group, so the ring overlaps Q / local-attention compute.

v4 changes vs v3 (trace-driven):
 - Split collective (K first, then V) issued ~30us earlier; remote
   unpack batched into 2 big DMAs on the idle Sync queue (was 20
   serialized gpsimd descriptors).
 - All bias matmuls removed: QKV bias folded into the PSUM-evac
   activation (per-partition bias), V bias folded host-side into the
   Wo bias (softmax rows sum to 1), Wo/W2 biases added to the residual
   stream via partition-broadcast + tensor_tensor on DVE.
 - Softmax denominators use reciprocal_approx_fast (5x).
 - Schraudolph-exp bitcast copies moved to GpSimd; more exp tiles
   moved off ScalarE.
 - W2 epilogue staggered per chain; output DMAs spread across queues.
"""

import sys

if "/opt/trn_rl_repo" not in sys.path:
    sys.path.insert(0, "/opt/trn_rl_repo")

import numpy as np

B, S, D = 2, 2048, 1024
H, DH, FF = 16, 64, 4096
NCORES = 8
TOK = (B * S) // NCORES      # 512 tokens per core
P = 128
TT = TOK // P                # 4 token tiles
KD = D // P                  # 8 contract tiles over D
FT = FF // P                 # 32 tiles over FF
GS = 4                       # group size (cores per batch)
NKJ = S // P                 # 16 key tiles per batch
NPR = H // 2                 # 8 head pairs
GROUPS = [[0, 1, 2, 3], [4, 5, 6, 7]]
KELEMS = KD * P * TOK        # fp8 elements per bounce region (K or V)

# Schraudolph exp: exp(x) ~= bitcast_f32(int32(x*A + B)); A folds the
# 1/sqrt(DH) score scale.
EXP_A = 12102203.161561485 * 0.125
EXP_B = 1064986823.0
DVE_EXP_JL = frozenset((1, 3))            # local-pass j tiles on VectorE
DVE_EXP_JR = frozenset((2, 5, 8, 11, 14))  # gathered-pass j tiles on VectorE

_cache = {}


def _prep(inputs):
    """Host-side: fold LN affines + V bias into weights, cast/arrange."""
    import ml_dtypes

    BF = ml_dtypes.bfloat16
    f32 = {k: np.asarray(v, dtype=np.float32) for k, v in inputs.items()}

    wqkv = f32["Wqkv"] * f32["ln1_g"][:, None]
    bqkv = f32["bqkv"] + f32["ln1_b"] @ f32["Wqkv"]
    w1 = f32["W1"] * f32["ln2_g"][:, None]
    b1 = f32["b1"] + f32["ln2_b"] @ f32["W1"]
    # softmax rows sum to 1, so attn(v + bv) = attn(v) + bv; fold the V
    # bias through Wo into the Wo bias.
    bo_eff = f32["bo"] + bqkv[2 * D:] @ f32["Wo"]

    def colmajor(w, nk):
        # [nk*P, M] -> [P, nk, M]
        return np.ascontiguousarray(
            w.reshape(nk, P, w.shape[1]).transpose(1, 0, 2))

    w1cm = colmajor(w1, KD)                       # [P, KD, FF]
    w1ch = np.ascontiguousarray(                  # [8, P, KD, 512]
        w1cm.reshape(P, KD, 8, 512).transpose(2, 0, 1, 3)).astype(BF)
    w2cm = colmajor(f32["W2"], FT)                # [P, FT, D]
    w2ch = np.ascontiguousarray(                  # [8, P, 4, D]
        w2cm.reshape(P, 8, 4, D).transpose(1, 0, 2, 3)).astype(BF)

    wk = {
        "wqk": colmajor(wqkv[:, :2 * D], KD).astype(BF),
        "wv": colmajor(wqkv[:, 2 * D:], KD).astype(BF),
        "wo": colmajor(f32["Wo"], KD).astype(BF),
        "w1ch": w1ch,
        "w2ch": w2ch,
        # qk bias as a per-partition column per m-tile: [P, 16]
        "bqkcol2": np.ascontiguousarray(
            bqkv[:2 * D].reshape(2 * KD, P).transpose(1, 0)).astype(
                np.float32),
        "borow": np.ascontiguousarray(bo_eff[None, :]).astype(BF),
        "b2row": np.ascontiguousarray(f32["b2"][None, :]).astype(BF),
        "b1col": np.ascontiguousarray(
            b1.reshape(FT, P).transpose(1, 0)).astype(np.float32),
    }
    x = np.ascontiguousarray(f32["x"]).reshape(B * S, D).astype(BF)
    return x, wk


def prepare_in_maps(inputs):
    x, wk = _prep(inputs)
    in_maps = []
    for c in range(NCORES):
        rank = c % GS
        # exp-bias masks: kill the own-rank key tiles in the gathered pass
        # (their true contribution comes from the local stash instead)
        mask_act = np.zeros((P, NKJ), np.float32)
        mask_dve = np.full((P, NKJ), EXP_B, np.float32)
        mask_act[:, rank * TT:(rank + 1) * TT] = -80.0
        mask_dve[:, rank * TT:(rank + 1) * TT] = EXP_B - 80.0 * 12102203.16
        m = {"x": np.ascontiguousarray(x[c * TOK:(c + 1) * TOK]),
             "mask_act": mask_act, "mask_dve": mask_dve}
        m.update(wk)
        in_maps.append(m)
    return in_maps


def _build():
    from contextlib import ExitStack
    from concourse import bacc, bass, tile, mybir
    from concourse.masks import make_identity

    F32 = mybir.dt.float32
    BF16 = mybir.dt.bfloat16
    F8 = mybir.dt.float8e4
    I32 = mybir.dt.int32
    Alu = mybir.AluOpType
    Act = mybir.ActivationFunctionType

    nc = bacc.Bacc("TRN2", target_bir_lowering=False, debug=False,
                   num_devices=NCORES)

    x_ext = nc.dram_tensor("x", [TOK, D], BF16, kind="ExternalInput")
    wqk_ext = nc.dram_tensor("wqk", [P, KD, 2 * D], BF16, kind="ExternalInput")
    wv_ext = nc.dram_tensor("wv", [P, KD, D], BF16, kind="ExternalInput")
    wo_ext = nc.dram_tensor("wo", [P, KD, D], BF16, kind="ExternalInput")
    w1_ext = nc.dram_tensor("w1ch", [8, P, KD, 512], BF16,
                            kind="ExternalInput")
    w2_ext = nc.dram_tensor("w2ch", [8, P, 4, D], BF16, kind="ExternalInput")
    bqkcol2_ext = nc.dram_tensor("bqkcol2", [P, 2 * KD], F32,
                                 kind="ExternalInput")
    borow_ext = nc.dram_tensor("borow", [1, D], BF16, kind="ExternalInput")
    b2row_ext = nc.dram_tensor("b2row", [1, D], BF16, kind="ExternalInput")
    b1col_ext = nc.dram_tensor("b1col", [P, FT], F32, kind="ExternalInput")
    mact_ext = nc.dram_tensor("mask_act", [P, NKJ], F32, kind="ExternalInput")
    mdve_ext = nc.dram_tensor("mask_dve", [P, NKJ], F32, kind="ExternalInput")
    out_ext = nc.dram_tensor("out", [TOK, D], F32, kind="ExternalOutput")

    with tile.TileContext(nc) as tc, ExitStack() as ctx:
        const = ctx.enter_context(tc.tile_pool(name="const", bufs=1))
        persist = ctx.enter_context(tc.tile_pool(name="persist", bufs=1))
        act = ctx.enter_context(tc.tile_pool(name="act", bufs=2))
        act1 = ctx.enter_context(tc.tile_pool(name="act1", bufs=1))
        probsp = ctx.enter_context(tc.tile_pool(name="probsp", bufs=4))
        mm_ps = ctx.enter_context(
            tc.tile_pool(name="mm_ps", bufs=2, space="PSUM"))
        av_ps = ctx.enter_context(
            tc.tile_pool(name="av_ps", bufs=2, space="PSUM"))
        dram = ctx.enter_context(tc.tile_pool(name="dram", bufs=1,
                                              space="DRAM"))

        # ---------------- input DMAs ----------------
        x1_sb = persist.tile([P, TT, D], BF16, tag="x1")
        for th in range(2):
            nc.sync.dma_start(
                x1_sb[:, th * 2:(th + 1) * 2, :],
                x_ext[th * 2 * P:(th + 1) * 2 * P, :].rearrange(
                    "(t p) d -> p t d", p=P))
        wqk_sb = persist.tile([P, KD, 2 * D], BF16, tag="wqk_g1T")
        # K-half of Wqk first so the K GEMM (and thus the K AllGather)
        # starts as early as possible; Q-half on another queue.
        nc.scalar.dma_start(wqk_sb[:, :, D:2 * D], wqk_ext[:, :, D:2 * D])
        nc.gpsimd.dma_start(wqk_sb[:, :, 0:D], wqk_ext[:, :, 0:D])
        wv_sb = persist.tile([P, KD, D], BF16, tag="wv_wo")
        nc.sync.dma_start(wv_sb[:], wv_ext[:])

        # ---------------- constants ----------------
        eps_t = const.tile([P, 1], F32)
        nc.vector.memset(eps_t[:], 1e-5)
        ones = const.tile([1, TOK], BF16)
        nc.vector.memset(ones[:], 1.0)
        ident = const.tile([P, P], BF16)
        make_identity(nc, ident[:])
        bqkcol2 = const.tile([P, 2 * KD], F32)
        nc.scalar.dma_start(bqkcol2[:], bqkcol2_ext[:])
        borow = const.tile([1, D], BF16)
        nc.scalar.dma_start(borow[:], borow_ext[:])
        b2row = const.tile([1, D], BF16)
        nc.scalar.dma_start(b2row[:], b2row_ext[:])
        b1col = const.tile([P, FT], F32)
        nc.scalar.dma_start(b1col[:], b1col_ext[:])
        mact_sb = const.tile([P, NKJ], F32)
        nc.sync.dma_start(mact_sb[:], mact_ext[:])
        mdve_sb = const.tile([P, NKJ], F32)
        nc.sync.dma_start(mdve_sb[:], mdve_ext[:])
        # broadcast bias rows for the residual adds (gpsimd is idle now)
        bo_bc = const.tile([P, D], BF16)
        nc.gpsimd.partition_broadcast(bo_bc[:], borow[:])
        b2_bc = const.tile([P, D], BF16)
        nc.gpsimd.partition_broadcast(b2_bc[:], b2row[:])

        # ---------------- helpers ----------------
        def layer_norm_all(src_tile, apply_fn):
            # stats for all TT tiles, one batched sqrt/recip, then apply
            mv = act.tile([P, TT, 2], F32, tag="ln_mv", name="ln_mv")
            for t in range(TT):
                stats = act.tile([P, 2, 6], F32, tag="ln_stats",
                                 name="ln_stats")
                nc.vector.bn_stats(stats[:, 0, :], src_tile[:, t, 0:512])
                nc.vector.bn_stats(stats[:, 1, :], src_tile[:, t, 512:1024])
                nc.vector.bn_aggr(mv[:, t, :], stats[:])
            rs = act.tile([P, TT], F32, tag="ln_rs", name="ln_rs")
            nc.scalar.activation(rs[:], mv[:, :, 1], Act.Sqrt,
                                 bias=eps_t[:])
            nc.vector.reciprocal(rs[:], rs[:])
            for t in range(TT):
                ht = act.tile([P, D], BF16, tag="hmt", name="hmt")
                nc.vector.tensor_scalar(ht[:], src_tile[:, t, :],
                                        scalar1=mv[:, t, 0:1],
                                        scalar2=rs[:, t:t + 1],
                                        op0=Alu.subtract, op1=Alu.mult)
                apply_fn(t, ht)

        def pe_transpose(dst_ap, src_ap):
            tp = mm_ps.tile([P, P], BF16, tag="mm2", name="tp_ps")
            nc.tensor.transpose(tp[:], src_ap, ident[:])
            nc.vector.tensor_copy(dst_ap, tp[:])

        def dve_exp(probs_ap_flat, sp_ap_flat, bconst):
            # Schraudolph: int mul-add on DVE, bitcast+fp8 cast on GpSimd
            ei = act1.tile([P, 2 * TOK], I32, tag="expi", name="expi")
            nc.vector.tensor_scalar(ei[:], sp_ap_flat,
                                    scalar1=EXP_A, scalar2=bconst,
                                    op0=Alu.mult, op1=Alu.add)
            nc.gpsimd.tensor_copy(probs_ap_flat, ei[:].bitcast(F32))

        # ---------------- phase 1: LN1 + transpose ----------------
        hT = persist.tile([P, KD, TOK], BF16, tag="actT")

        def _ln1_apply(t, ht):
            for k in range(KD):
                pe_transpose(hT[:, k, t * P:(t + 1) * P],
                             ht[:, k * P:(k + 1) * P])

        layer_norm_all(x1_sb, _ln1_apply)
        # residual picks up the (folded) Wo bias here, before Wo's add
        for t in range(TT):
            nc.vector.tensor_tensor(x1_sb[:, t, :], x1_sb[:, t, :],
                                    bo_bc[:], op=Alu.add)

        # ---------------- phase 2: K, CC-K, V, CC-V, Q ----------------
        qT = persist.tile([P, KD, TOK], F8, tag="qT")
        kT_loc = persist.tile([P, KD, TOK], F8, tag="kTloc")

        def qk_block(mp, is_k):
            ps = mm_ps.tile([P, 2, TOK], F32, tag="mm2", name="mm_qkv")
            for hf in range(2):
                m = 2 * mp + hf
                for k in range(KD):
                    nc.tensor.matmul(ps[:, hf, :],
                                     wqk_sb[:, k, m * P:(m + 1) * P],
                                     hT[:, k, :],
                                     start=(k == 0), stop=(k == KD - 1))
            for hf in range(2):
                m = 2 * mp + hf
                dst = kT_loc[:, m - 8, :] if is_k else qT[:, m, :]
                nc.scalar.activation(dst, ps[:, hf, :], Act.Identity,
                                     bias=bqkcol2[:, m:m + 1])

        for mp in range(4, 8):          # K first
            qk_block(mp, is_k=True)

        # K bounce + AllGather (starts while V/Q still compute)
        cc_in_k = dram.tile([KELEMS], F8)
        cc_out_k = dram.tile([GS * KELEMS], F8)
        nc.gpsimd.dma_start(
            cc_in_k[:].rearrange("(k p t) -> p k t", k=KD, p=P),
            kT_loc[:])
        nc.gpsimd.collective_compute(
            "AllGather", Alu.bypass, ins=[cc_in_k[:]], outs=[cc_out_k[:]],
            replica_groups=GROUPS)

        # V (natural layout) -> v_loc (local AV operand + bounce source)
        v_loc = persist.tile([P, TT, H, 65], F8, tag="vloc")
        v_rem = persist.tile([P, GS, H, TT, 65], F8, tag="vrem")
        nc.vector.memset(v_loc[:, :, :, 64:65], 1.0)
        nc.vector.memset(v_rem[:, :, :, :, 64:65], 1.0)
        for c in range(2):
            pss = [mm_ps.tile([P, 2, 512], F32, tag="mm2", name="mm_v")
                   for _ in range(2)]
            for k in range(KD):
                for t in range(TT):
                    nc.tensor.matmul(pss[t // 2][:, t % 2, :],
                                     hT[:, k, t * P:(t + 1) * P],
                                     wv_sb[:, k, c * 512:(c + 1) * 512],
                                     start=(k == 0), stop=(k == KD - 1))
            for t in range(TT):
                nc.vector.tensor_copy(
                    v_loc[:, t, c * 8:(c + 1) * 8, 0:64],
                    pss[t // 2][:, t % 2, :].rearrange(
                        "p (h f) -> p h f", h=8))

        # V bounce + AllGather
        cc_in_v = dram.tile([KELEMS], F8)
        cc_out_v = dram.tile([GS * KELEMS], F8)
        for t in range(TT):
            nc.gpsimd.dma_start(
                cc_in_v[t * P * D:(t + 1) * P * D].rearrange(
                    "(p h f) -> p h f", p=P, h=H),
                v_loc[:, t, :, 0:64])
        nc.gpsimd.collective_compute(
            "AllGather", Alu.bypass, ins=[cc_in_v[:]], outs=[cc_out_v[:]],
            replica_groups=GROUPS)

        # Q overlaps the rings
        for mp in range(0, 4):
            qk_block(mp, is_k=False)

        # weight prefetch during the rings
        wo_sb = persist.tile([P, KD, D], BF16, tag="wv_wo")
        nc.scalar.dma_start(wo_sb[:], wo_ext[:])

        # batched remote unpack on the (idle) Sync queue; these wait on
        # the collectives' completion sems without blocking compute
        kT_rem = persist.tile([P, KD, GS, TOK], F8, tag="ktrem")
        for r in range(GS):
            nc.sync.dma_start(
                kT_rem[:, :, r, :],
                cc_out_k[r * KELEMS:(r + 1) * KELEMS].rearrange(
                    "(k p t) -> p k t", k=KD, p=P))
        for r in range(GS):
            nc.sync.dma_start(
                v_rem[:, r, :, :, 0:64],
                cc_out_v[r * KELEMS:(r + 1) * KELEMS].rearrange(
                    "(t p h f) -> p h t f", t=TT, p=P, h=H))

        # local attention pass during the rings: probs for own 4 key tiles
        probs_loc = persist.tile([P, NPR, TT, 2, TOK], F8, tag="ploc")
        for pr in range(NPR):
            for jl in range(TT):
                sp = mm_ps.tile([P, 2, TOK], F32, tag="mm2", name="mm_scl")
                for hp in range(2):
                    lo = hp * 64
                    nc.tensor.matmul(
                        sp[:, hp, :],
                        kT_loc[lo:lo + 64, pr, jl * P:(jl + 1) * P],
                        qT[lo:lo + 64, pr, :], start=True, stop=True)
                pl = probs_loc[:, pr, jl, :, :]
                if jl in DVE_EXP_JL:
                    dve_exp(pl.rearrange("p a b -> p (a b)"),
                            sp[:].rearrange("p a b -> p (a b)"), EXP_B)
                else:
                    nc.scalar.activation(pl, sp[:], Act.Exp, scale=0.125)

        # W1 stream chunks: manual double-buffer via two tags. Only the
        # first three are prefetched here; the rest are emitted just-in-time
        # inside the W1 loop AFTER their buffer's previous readers, so the
        # WAR dependency is correctly formed.
        w1c = []
        for ch in range(3):
            t_ = persist.tile([P, KD, 512], BF16, tag=f"ws{ch % 3}",
                              name=f"w1c{ch}")
            nc.sync.dma_start(t_[:], w1_ext[ch])
            w1c.append(t_)

        # ------- phase 3: attention (V-stationary AV -> attnT) -------
        attnT = persist.tile([P, KD, TOK], BF16, tag="attnT")

        for pr in range(NPR):
            av = av_ps.tile([P, 2, TOK], F32, tag="av", name="av")
            # local tiles from the stash
            for jl in range(TT):
                for hp in range(2):
                    h = 2 * pr + hp
                    nc.tensor.matmul(
                        av[0:65, hp, :], v_loc[:, jl, h, :],
                        probs_loc[:, pr, jl, hp, :],
                        start=(jl == 0), stop=False)
            # gathered tiles (all four slots; own slot masked to zero).
            # AV for tile j is emitted two iterations behind its scores so
            # the in-order PE never waits out the exp latency.
            pending = []

            def emit_av(jr, probs):
                for hp in range(2):
                    h = 2 * pr + hp
                    ri, jj = divmod(jr, TT)
                    nc.tensor.matmul(
                        av[0:65, hp, :], v_rem[:, ri, h, jj, :],
                        probs[:, hp, :],
                        start=False, stop=(jr == GS * TT - 1))

            for jr in range(GS * TT):
                ri, jj = divmod(jr, TT)
                sp = mm_ps.tile([P, 2, TOK], F32, tag="mm2", name="mm_sc")
                for hp in range(2):
                    lo = hp * 64
                    nc.tensor.matmul(
                        sp[:, hp, :],
                        kT_rem[lo:lo + 64, pr, ri, jj * P:(jj + 1) * P],
                        qT[lo:lo + 64, pr, :], start=True, stop=True)
                probs = probsp.tile([P, 2, TOK], F8, tag="probs",
                                    name="probs")
                if jr in DVE_EXP_JR:
                    dve_exp(probs[:].rearrange("p a b -> p (a b)"),
                            sp[:].rearrange("p a b -> p (a b)"),
                            mdve_sb[:, jr:jr + 1])
                else:
                    nc.scalar.activation(probs[:], sp[:], Act.Exp,
                                         scale=0.125,
                                         bias=mact_sb[:, jr:jr + 1])
                pending.append((jr, probs))
                if len(pending) > 2:
                    emit_av(*pending.pop(0))
            while pending:
                emit_av(*pending.pop(0))
            # evacuate PSUM with plain copies (av's only readers), so the
            # next pair's AV chain never waits on the reciprocal chain;
            # then normalize from the SBUF copies, multiplying in place.
            dens = [act1.tile([1, TOK], F32, tag=f"den{hp}", name="den")
                    for hp in range(2)]
            for hp in range(2):
                nc.vector.tensor_copy(attnT[hp * 64:(hp + 1) * 64, pr, :],
                                      av[0:64, hp, :])
                nc.vector.tensor_copy(dens[hp][:], av[64:65, hp, :])
            for hp in range(2):
                rows = attnT[hp * 64:(hp + 1) * 64, pr, :]
                rec = act1.tile([1, TOK], F32, tag="arec", name="arec")
                nc.vector.reciprocal_approx_fast(rec[:], dens[hp][:])
                rbc = act1.tile([P, TOK], F32, tag="abc", name="abc")
                nc.gpsimd.partition_broadcast(rbc[:], rec[:])
                nc.vector.tensor_tensor(
                    rows, rows, rbc[hp * 64:(hp + 1) * 64, :], op=Alu.mult)

        # ---------------- phase 4: Wo (full PSUM accumulation) --------
        for c in range(2):
            for qth in range(2):
                ps = mm_ps.tile([P, 2, 512], F32, tag="mm2", name="mm_wo")
                for q2 in range(2):
                    qt = 2 * qth + q2
                    for pr in range(NPR):
                        nc.tensor.matmul(
                            ps[:, q2, :],
                            attnT[:, pr, qt * P:(qt + 1) * P],
                            wo_sb[:, pr, c * 512:(c + 1) * 512],
                            start=(pr == 0), stop=(pr == NPR - 1))
                for q2 in range(2):
                    qt = 2 * qth + q2
                    sl = x1_sb[:, qt, c * 512:(c + 1) * 512]
                    nc.vector.tensor_add(sl, sl, ps[:, q2, :])

        # ---------------- phase 5: LN2 + transpose ----------------
        mT = persist.tile([P, KD, TOK], BF16, tag="actT")

        def _ln2_apply(t, mt):
            for k in range(KD):
                pe_transpose(mT[:, k, t * P:(t + 1) * P],
                             mt[:, k * P:(k + 1) * P])

        layer_norm_all(x1_sb, _ln2_apply)
        # residual picks up the W2 bias here (after LN2 consumed x1)
        for t in range(TT):
            nc.vector.tensor_tensor(x1_sb[:, t, :], x1_sb[:, t, :],
                                    b2_bc[:], op=Alu.add)

        # ---------------- phase 6: W1 + gelu ----------------
        g1T = persist.tile([P, FT, TOK], BF16, tag="wqk_g1T")
        for mp in range(FT // 2):
            if mp % 2 == 0 and 2 <= mp and mp // 2 + 2 < 8:
                # refill chunk mp//2+2: aliases chunk mp//2-1, whose
                # readers (mp-2, mp-1) are already emitted
                ch = mp // 2 + 2
                t_ = persist.tile([P, KD, 512], BF16, tag=f"ws{ch % 3}",
                                  name=f"w1c{ch}")
                nc.sync.dma_start(t_[:], w1_ext[ch])
                w1c.append(t_)
            wt = w1c[mp // 2]
            mo = mp % 2
            ps = mm_ps.tile([P, 2, TOK], F32, tag="mm2", name="mm_w1")
            for hf in range(2):
                for k in range(KD):
                    nc.tensor.matmul(ps[:, hf, :],
                                     wt[:, k, (2 * mo + hf) * P:
                                        (2 * mo + hf + 1) * P],
                                     mT[:, k, :],
                                     start=(k == 0), stop=(k == KD - 1))
            for hf in range(2):
                m = 2 * mp + hf
                nc.scalar.activation(g1T[:, m, :], ps[:, hf, :],
                                     Act.Gelu_apprx_tanh,
                                     bias=b1col[:, m:m + 1])

        # ---------------- phase 7: W2 (8 parallel chains) ------------
        # 8 chains (c, qt) in 4 PSUM tiles: 2 from mm_ps + 2 from av_ps.
        w2c = []
        for ch in range(3):
            t_ = persist.tile([P, 4, D], BF16, tag=f"ws{ch % 3}",
                              name=f"w2c{ch}")
            nc.sync.dma_start(t_[:], w2_ext[ch])
            w2c.append(t_)

        pss = [mm_ps.tile([P, 2, 512], F32, tag="mm2", name="mm_w2")
               for _ in range(2)]
        pss += [av_ps.tile([P, 2, 512], F32, tag="av", name="mm_w2b")
                for _ in range(2)]

        def chain(c, qt):
            t_ = pss[c * 2 + qt // 2]
            return t_[:, qt % 2, :]

        out_q = [nc.scalar, nc.sync, nc.gpsimd, nc.sync]

        def finish_chain(c, qt, qi):
            ot = act1.tile([P, 512], F32, tag=f"oout{qt % 2}", name="oout")
            nc.vector.scalar_tensor_tensor(
                ot[:], chain(c, qt), 1.0,
                x1_sb[:, qt, c * 512:(c + 1) * 512],
                op0=Alu.mult, op1=Alu.add)
            out_q[qi % 4].dma_start(
                out_ext[qt * P:(qt + 1) * P, c * 512:(c + 1) * 512],
                ot[:])

        for ch in range(8):
            if 1 <= ch and ch + 2 < 8:
                # refill chunk ch+2: aliases ch-1, whose readers are emitted
                t_ = persist.tile([P, 4, D], BF16, tag=f"ws{(ch + 2) % 3}",
                                  name=f"w2c{ch + 2}")
                nc.sync.dma_start(t_[:], w2_ext[ch + 2])
                w2c.append(t_)
            if ch < 7:
                for fl in range(4):
                    ff = ch * 4 + fl
                    for c in range(2):
                        for qt in range(TT):
                            nc.tensor.matmul(
                                chain(c, qt),
                                g1T[:, ff, qt * P:(qt + 1) * P],
                                w2c[ch][:, fl, c * 512:(c + 1) * 512],
                                start=(ff == 0), stop=False)
            else:
                # last chunk: finish chain-by-chain so the evacuations
                # pipeline with the remaining matmuls instead of
                # serializing after the final one
                qi = 0
                for c in range(2):
                    for qt in range(TT):
                        for fl in range(4):
                            ff = ch * 4 + fl
                            nc.tensor.matmul(
                                chain(c, qt),
                                g1T[:, ff, qt * P:(qt + 1) * P],
                                w2c[ch][:, fl, c * 512:(c + 1) * 512],
                                start=False, stop=(fl == 3))
                        finish_chain(c, qt, qi)
                        qi += 1

    nc.compile()
    return nc


def _get_nc():
    if "nc" not in _cache:
        _cache["nc"] = _build()
    return _cache["nc"]


def kernel(**inputs):
    from concourse.bass_utils import run_bass_kernel_spmd

    nc = _get_nc()
    in_maps = prepare_in_maps(inputs)
    res = run_bass_kernel_spmd(nc, in_maps, core_ids=list(range(NCORES)))
    out = np.concatenate([res.results[c]["out"] for c in range(NCORES)],
                         axis=0)
    return out.reshape(B, S, D).astype(np.float32)


# revision 15
# speedup vs baseline: 1.3500x; 1.3500x over previous
"""Distributed Trainium2 kernel for a dense transformer block (v5).

Sharding: sequence-parallel over the 8 NeuronCores. The flattened
[B*S=4096, D=1024] token stream is split into 8 contiguous shards of 512
tokens (cores 0-3 hold batch 0, cores 4-7 hold batch 1). Weights are
replicated. Collectives: an AllGather of K^T right after the K GEMM and
a second AllGather of V right after the V GEMM, both within the 4-core
batch group, so the rings overlap Q / local-attention compute.

v5 highlights (trace-driven):
 - Softmax denominators come from a col-tiled ones-matmul into PSUM
   partitions 64..127 that runs CONCURRENTLY with the V matmul
   (cols 0..63) - measured 0ns for the second matmul of each pair.
   Normalization is then reciprocal_approx_fast([64,512]) + one
   tensor_tensor, killing the old dens-copy/broadcast/reciprocal chain.
 - Gathered-pass probs are bf16: ScalarE exp writes bf16; the VectorE
   share uses a ONE-op Schraudolph (int16 output = top 16 bits of the
   f32 trick, bitcast to bf16). AV matmuls mix fp8 V x bf16 probs
   (validated exact on HW).
 - V is stored 64-wide/contiguous (no interleaved ones column), so the
   bounce is one DMA and the remote unpack runs at 256B granularity.
 - Split collectives (K, then V), batched unpacks on the Sync queue,
   no bias matmuls, W2 epilogue staggered per chain.
"""

import sys

if "/opt/trn_rl_repo" not in sys.path:
    sys.path.insert(0, "/opt/trn_rl_repo")

import numpy as np

B, S, D = 2, 2048, 1024
H, DH, FF = 16, 64, 4096
NCORES = 8
TOK = (B * S) // NCORES      # 512 tokens per core
P = 128
TT = TOK // P                # 4 token tiles
KD = D // P                  # 8 contract tiles over D
FT = FF // P                 # 32 tiles over FF
GS = 4                       # group size (cores per batch)
NKJ = S // P                 # 16 key tiles per batch
NPR = H // 2                 # 8 head pairs
GROUPS = [[0, 1, 2, 3], [4, 5, 6, 7]]
KELEMS = KD * P * TOK        # fp8 elements per bounce region (K or V)

# Schraudolph exp: exp(x) ~= bitcast_f32(int32(x*A + B)); A folds the
# 1/sqrt(DH) score scale. The /65536 variants produce the TOP 16 bits
# directly as an int16, which bitcast as bf16.
EXP_AF = 12102203.161561485
EXP_A = EXP_AF * 0.125
EXP_B = 1064986823.0
EXP_A16 = EXP_A / 65536.0
EXP_B16 = EXP_B / 65536.0
DVE_EXP_JL = frozenset((1, 3))             # local-pass j tiles on VectorE
DVE_EXP_JR = frozenset((2, 5, 8, 11, 14))  # gathered-pass j tiles on VectorE

_cache = {}
DEBUG = False


def _prep(inputs):
    """Host-side: fold LN affines + V bias into weights, cast/arrange."""
    import ml_dtypes

    BF = ml_dtypes.bfloat16
    f32 = {k: np.asarray(v, dtype=np.float32) for k, v in inputs.items()}

    wqkv = f32["Wqkv"] * f32["ln1_g"][:, None]
    bqkv = f32["bqkv"] + f32["ln1_b"] @ f32["Wqkv"]
    w1 = f32["W1"] * f32["ln2_g"][:, None]
    b1 = f32["b1"] + f32["ln2_b"] @ f32["W1"]
    # softmax rows sum to 1, so attn(v + bv) = attn(v) + bv; fold the V
    # bias through Wo into the Wo bias.
    bo_eff = f32["bo"] + bqkv[2 * D:] @ f32["Wo"]

    def colmajor(w, nk):
        # [nk*P, M] -> [P, nk, M]
        return np.ascontiguousarray(
            w.reshape(nk, P, w.shape[1]).transpose(1, 0, 2))

    w1cm = colmajor(w1, KD)                       # [P, KD, FF]
    w1ch = np.ascontiguousarray(                  # [8, P, KD, 512]
        w1cm.reshape(P, KD, 8, 512).transpose(2, 0, 1, 3)).astype(BF)
    w2cm = colmajor(f32["W2"], FT)                # [P, FT, D]
    w2ch = np.ascontiguousarray(                  # [8, P, 4, D]
        w2cm.reshape(P, 8, 4, D).transpose(1, 0, 2, 3)).astype(BF)

    wk = {
        "wqk": colmajor(wqkv[:, :2 * D], KD).astype(BF),
        "wv": colmajor(wqkv[:, 2 * D:], KD).astype(BF),
        "wo": colmajor(f32["Wo"], KD).astype(BF),
        "w1ch": w1ch,
        "w2ch": w2ch,
        # qk bias as a per-partition column per m-tile: [P, 16]
        "bqkcol2": np.ascontiguousarray(
            bqkv[:2 * D].reshape(2 * KD, P).transpose(1, 0)).astype(
                np.float32),
        "borow": np.ascontiguousarray(bo_eff[None, :]).astype(BF),
        "b2row": np.ascontiguousarray(f32["b2"][None, :]).astype(BF),
        "b1col": np.ascontiguousarray(
            b1.reshape(FT, P).transpose(1, 0)).astype(np.float32),
    }
    x = np.ascontiguousarray(f32["x"]).reshape(B * S, D).astype(BF)
    return x, wk


def prepare_in_maps(inputs):
    x, wk = _prep(inputs)
    in_maps = []
    for c in range(NCORES):
        rank = c % GS
        # exp-bias masks: kill the own-rank key tiles in the gathered pass
        # (their true contribution comes from the local stash instead)
        mask_act = np.zeros((P, NKJ), np.float32)
        mask_dve = np.full((P, NKJ), EXP_B16, np.float32)
        mask_act[:, rank * TT:(rank + 1) * TT] = -80.0
        mask_dve[:, rank * TT:(rank + 1) * TT] = \
            (EXP_B - 80.0 * EXP_AF) / 65536.0
        m = {"x": np.ascontiguousarray(x[c * TOK:(c + 1) * TOK]),
             "mask_act": mask_act, "mask_dve": mask_dve}
        m.update(wk)
        in_maps.append(m)
    return in_maps


def _build():
    from contextlib import ExitStack
    from concourse import bacc, bass, tile, mybir
    from concourse.masks import make_identity

    F32 = mybir.dt.float32
    BF16 = mybir.dt.bfloat16
    F8 = mybir.dt.float8e4
    I16 = mybir.dt.int16
    I32 = mybir.dt.int32
    Alu = mybir.AluOpType
    Act = mybir.ActivationFunctionType

    nc = bacc.Bacc("TRN2", target_bir_lowering=False, debug=False,
                   num_devices=NCORES)

    x_ext = nc.dram_tensor("x", [TOK, D], BF16, kind="ExternalInput")
    wqk_ext = nc.dram_tensor("wqk", [P, KD, 2 * D], BF16, kind="ExternalInput")
    wv_ext = nc.dram_tensor("wv", [P, KD, D], BF16, kind="ExternalInput")
    wo_ext = nc.dram_tensor("wo", [P, KD, D], BF16, kind="ExternalInput")
    w1_ext = nc.dram_tensor("w1ch", [8, P, KD, 512], BF16,
                            kind="ExternalInput")
    w2_ext = nc.dram_tensor("w2ch", [8, P, 4, D], BF16, kind="ExternalInput")
    bqkcol2_ext = nc.dram_tensor("bqkcol2", [P, 2 * KD], F32,
                                 kind="ExternalInput")
    borow_ext = nc.dram_tensor("borow", [1, D], BF16, kind="ExternalInput")
    b2row_ext = nc.dram_tensor("b2row", [1, D], BF16, kind="ExternalInput")
    b1col_ext = nc.dram_tensor("b1col", [P, FT], F32, kind="ExternalInput")
    mact_ext = nc.dram_tensor("mask_act", [P, NKJ], F32, kind="ExternalInput")
    mdve_ext = nc.dram_tensor("mask_dve", [P, NKJ], F32, kind="ExternalInput")
    out_ext = nc.dram_tensor("out", [TOK, D], F32, kind="ExternalOutput")
    dbg = {}
    if DEBUG:
        for nm, shp, dt in [
                ("d_hT", [P, KD, TOK], BF16), ("d_kTloc", [P, KD, TOK], F8),
                ("d_qT", [P, KD, TOK], F8), ("d_vloc", [P, H, TT * 64], F8),
                ("d_ktrem", [P, KD, GS, TOK], F8),
                ("d_vrem", [P, GS, H, TT * 64], F8),
                ("d_ploc", [P, NPR, TT, 2, TOK], F8),
                ("d_attnT", [P, KD, TOK], BF16),
                ("d_x1a", [P, TT, D], BF16), ("d_g1T", [P, FT, TOK], BF16)]:
            dbg[nm] = nc.dram_tensor(nm, shp, dt, kind="ExternalOutput")

    with tile.TileContext(nc) as tc, ExitStack() as ctx:
        const = ctx.enter_context(tc.tile_pool(name="const", bufs=1))
        persist = ctx.enter_context(tc.tile_pool(name="persist", bufs=1))
        act = ctx.enter_context(tc.tile_pool(name="act", bufs=2))
        act1 = ctx.enter_context(tc.tile_pool(name="act1", bufs=1))
        probsp = ctx.enter_context(tc.tile_pool(name="probsp", bufs=3))
        mm_ps = ctx.enter_context(
            tc.tile_pool(name="mm_ps", bufs=2, space="PSUM"))
        av_ps = ctx.enter_context(
            tc.tile_pool(name="av_ps", bufs=2, space="PSUM"))
        dram = ctx.enter_context(tc.tile_pool(name="dram", bufs=1,
                                              space="DRAM"))

        # ---------------- input DMAs ----------------
        # x tile-by-tile so LN1 stats start after ~1/4 of the load
        x1_sb = persist.tile([P, TT, D], BF16, tag="x1")
        for th in range(TT):
            nc.sync.dma_start(
                x1_sb[:, th:th + 1, :],
                x_ext[th * P:(th + 1) * P, :].rearrange(
                    "(t p) d -> p t d", p=P))
        wqk_sb = persist.tile([P, KD, 2 * D], BF16, tag="wqk_g1T")
        # K-half of Wqk first so the K GEMM (and thus the K AllGather)
        # starts as early as possible; Q-half on another queue.
        nc.scalar.dma_start(wqk_sb[:, :, D:2 * D], wqk_ext[:, :, D:2 * D])
        nc.gpsimd.dma_start(wqk_sb[:, :, 0:D], wqk_ext[:, :, 0:D])
        wv_sb = persist.tile([P, KD, D], BF16, tag="wv_wo")
        nc.sync.dma_start(wv_sb[:], wv_ext[:])

        # ---------------- constants ----------------
        eps_t = const.tile([P, 1], F32)
        nc.vector.memset(eps_t[:], 1e-5)
        ones64 = const.tile([P, 64], F8)
        nc.vector.memset(ones64[:], 1.0)
        ident = const.tile([P, P], BF16)
        make_identity(nc, ident[:])
        bqkcol2 = const.tile([P, 2 * KD], F32)
        nc.scalar.dma_start(bqkcol2[:], bqkcol2_ext[:])
        borow = const.tile([1, D], BF16)
        nc.scalar.dma_start(borow[:], borow_ext[:])
        b2row = const.tile([1, D], BF16)
        nc.scalar.dma_start(b2row[:], b2row_ext[:])
        b1col = const.tile([P, FT], F32)
        nc.scalar.dma_start(b1col[:], b1col_ext[:])
        mact_sb = const.tile([P, NKJ], F32)
        nc.sync.dma_start(mact_sb[:], mact_ext[:])
        mdve_sb = const.tile([P, NKJ], F32)
        nc.sync.dma_start(mdve_sb[:], mdve_ext[:])
        # broadcast bias rows for the residual adds (gpsimd is idle now)
        bo_bc = const.tile([P, D], BF16)
        nc.gpsimd.partition_broadcast(bo_bc[:], borow[:])
        b2_bc = const.tile([P, D], BF16)
        nc.gpsimd.partition_broadcast(b2_bc[:], b2row[:])

        # ---------------- helpers ----------------
        def layer_norm_all(src_tile, apply_fn):
            # stats for all TT tiles, one batched sqrt/recip, then apply
            mv = act.tile([P, TT, 2], F32, tag="ln_mv", name="ln_mv")
            for t in range(TT):
                stats = act.tile([P, 2, 6], F32, tag="ln_stats",
                                 name="ln_stats")
                nc.vector.bn_stats(stats[:, 0, :], src_tile[:, t, 0:512])
                nc.vector.bn_stats(stats[:, 1, :], src_tile[:, t, 512:1024])
                nc.vector.bn_aggr(mv[:, t, :], stats[:])
            rs = act.tile([P, TT], F32, tag="ln_rs", name="ln_rs")
            nc.scalar.activation(rs[:], mv[:, :, 1], Act.Sqrt,
                                 bias=eps_t[:])
            nc.vector.reciprocal(rs[:], rs[:])
            for t in range(TT):
                ht = act.tile([P, D], BF16, tag="hmt", name="hmt")
                nc.vector.tensor_scalar(ht[:], src_tile[:, t, :],
                                        scalar1=mv[:, t, 0:1],
                                        scalar2=rs[:, t:t + 1],
                                        op0=Alu.subtract, op1=Alu.mult)
                apply_fn(t, ht)

        def pe_transpose(dst_ap, src_ap):
            tp = mm_ps.tile([P, P], BF16, tag="mm2", name="tp_ps")
            nc.tensor.transpose(tp[:], src_ap, ident[:])
            nc.vector.tensor_copy(dst_ap, tp[:])

        def dve_exp16(probs_bf16_flat, sp_ap_flat, bconst):
            # one-op Schraudolph: int16(x*A/65536 + B/65536) == top 16
            # bits of the f32 trick == the bf16 exp approximation
            nc.vector.tensor_scalar(probs_bf16_flat.bitcast(I16), sp_ap_flat,
                                    scalar1=EXP_A16, scalar2=bconst,
                                    op0=Alu.mult, op1=Alu.add)

        def dve_exp8(probs_f8_flat, sp_ap_flat):
            # two-op Schraudolph for the fp8 local stash
            ei = act1.tile([P, 2 * TOK], I32, tag="expi", name="expi")
            nc.vector.tensor_scalar(ei[:], sp_ap_flat,
                                    scalar1=EXP_A, scalar2=EXP_B,
                                    op0=Alu.mult, op1=Alu.add)
            nc.vector.tensor_copy(probs_f8_flat, ei[:].bitcast(F32))

        # ---------------- phase 1: LN1 + transpose ----------------
        hT = persist.tile([P, KD, TOK], BF16, tag="actT")

        def _ln1_apply(t, ht):
            for k in range(KD):
                pe_transpose(hT[:, k, t * P:(t + 1) * P],
                             ht[:, k * P:(k + 1) * P])

        layer_norm_all(x1_sb, _ln1_apply)

        # ---------------- phase 2: K, CC-K, V, CC-V, Q ----------------
        qT = persist.tile([P, KD, TOK], F8, tag="qT")
        kT_loc = persist.tile([P, KD, TOK], F8, tag="kTloc")

        def qk_block(mp, is_k):
            ps = mm_ps.tile([P, 2, TOK], F32, tag="mm2", name="mm_qkv")
            for hf in range(2):
                m = 2 * mp + hf
                for k in range(KD):
                    nc.tensor.matmul(ps[:, hf, :],
                                     wqk_sb[:, k, m * P:(m + 1) * P],
                                     hT[:, k, :],
                                     start=(k == 0), stop=(k == KD - 1))
            for hf in range(2):
                m = 2 * mp + hf
                dst = kT_loc[:, m - 8, :] if is_k else qT[:, m, :]
                nc.scalar.activation(dst, ps[:, hf, :], Act.Identity,
                                     bias=bqkcol2[:, m:m + 1])

        for mp in range(4, 8):          # K first
            qk_block(mp, is_k=True)

        # K bounce + AllGather (starts while V/Q still compute)
        cc_in_k = dram.tile([KELEMS], F8)
        cc_out_k = dram.tile([GS * KELEMS], F8)
        nc.gpsimd.dma_start(
            cc_in_k[:].rearrange("(k p t) -> p k t", k=KD, p=P),
            kT_loc[:])
        nc.gpsimd.collective_compute(
            "AllGather", Alu.bypass, ins=[cc_in_k[:]], outs=[cc_out_k[:]],
            replica_groups=GROUPS)

        # V: pure 64-wide layout [P, H, TT*64]; wire format (p, h, t*f)
        v_loc = persist.tile([P, H, TT * 64], F8, tag="vloc")
        v_rem = persist.tile([P, GS, H, TT * 64], F8, tag="vrem")
        for c in range(2):
            pss = [mm_ps.tile([P, 2, 512], F32, tag="mm2", name="mm_v")
                   for _ in range(2)]
            for k in range(KD):
                for t in range(TT):
                    nc.tensor.matmul(pss[t // 2][:, t % 2, :],
                                     hT[:, k, t * P:(t + 1) * P],
                                     wv_sb[:, k, c * 512:(c + 1) * 512],
                                     start=(k == 0), stop=(k == KD - 1))
            for t in range(TT):
                nc.vector.tensor_copy(
                    v_loc[:, c * 8:(c + 1) * 8, t * 64:(t + 1) * 64],
                    pss[t // 2][:, t % 2, :].rearrange(
                        "p (h f) -> p h f", h=8))

        # V bounce + AllGather (one contiguous DMA)
        cc_in_v = dram.tile([KELEMS], F8)
        cc_out_v = dram.tile([GS * KELEMS], F8)
        nc.gpsimd.dma_start(
            cc_in_v[:].rearrange("(p h f) -> p h f", p=P, h=H),
            v_loc[:])
        nc.gpsimd.collective_compute(
            "AllGather", Alu.bypass, ins=[cc_in_v[:]], outs=[cc_out_v[:]],
            replica_groups=GROUPS)

        # Q overlaps the rings
        for mp in range(0, 4):
            qk_block(mp, is_k=False)

        # weight prefetch during the rings
        wo_sb = persist.tile([P, KD, D], BF16, tag="wv_wo")
        nc.scalar.dma_start(wo_sb[:], wo_ext[:])

        # batched remote unpack on the (idle) Sync queue; these wait on
        # the collectives' completion sems without blocking compute
        kT_rem = persist.tile([P, KD, GS, TOK], F8, tag="ktrem")
        for r in range(GS):
            nc.sync.dma_start(
                kT_rem[:, :, r, :],
                cc_out_k[r * KELEMS:(r + 1) * KELEMS].rearrange(
                    "(k p t) -> p k t", k=KD, p=P))
        for r in range(GS):
            nc.sync.dma_start(
                v_rem[:, r, :, :],
                cc_out_v[r * KELEMS:(r + 1) * KELEMS].rearrange(
                    "(p h f) -> p h f", p=P, h=H))

        # residual picks up the (folded) Wo bias during the ring window
        for t in range(TT):
            nc.vector.tensor_tensor(x1_sb[:, t, :], x1_sb[:, t, :],
                                    bo_bc[:], op=Alu.add)

        # local attention pass during the rings: probs for own 4 key tiles
        probs_loc = persist.tile([P, NPR, TT, 2, TOK], F8, tag="ploc")
        for pr in range(NPR):
            for jl in range(TT):
                sp = mm_ps.tile([P, 2, TOK], F32, tag="mm2", name="mm_scl")
                for hp in range(2):
                    lo = hp * 64
                    nc.tensor.matmul(
                        sp[:, hp, :],
                        kT_loc[lo:lo + 64, pr, jl * P:(jl + 1) * P],
                        qT[lo:lo + 64, pr, :], start=True, stop=True)
                pl = probs_loc[:, pr, jl, :, :]
                if jl in DVE_EXP_JL:
                    dve_exp8(pl.rearrange("p a b -> p (a b)"),
                             sp[:].rearrange("p a b -> p (a b)"))
                else:
                    nc.scalar.activation(pl, sp[:], Act.Exp, scale=0.125)

        if DEBUG:
            nc.scalar.dma_start(dbg["d_hT"][:], hT[:])
            nc.scalar.dma_start(dbg["d_kTloc"][:], kT_loc[:])
            nc.scalar.dma_start(dbg["d_qT"][:], qT[:])
            nc.scalar.dma_start(dbg["d_vloc"][:], v_loc[:])
            nc.scalar.dma_start(dbg["d_ktrem"][:], kT_rem[:])
            nc.scalar.dma_start(dbg["d_vrem"][:], v_rem[:])
            nc.scalar.dma_start(dbg["d_ploc"][:], probs_loc[:])

        # W1 stream chunks: manual double-buffer via tags. Only the first
        # three are prefetched here; the rest are emitted just-in-time
        # inside the W1 loop AFTER their buffer's previous readers, so the
        # WAR dependency is correctly formed.
        w1c = []
        for ch in range(3):
            t_ = persist.tile([P, KD, 512], BF16, tag=f"ws{ch % 3}",
                              name=f"w1c{ch}")
            nc.sync.dma_start(t_[:], w1_ext[ch])
            w1c.append(t_)

        # ------- phase 3: attention (V-stationary AV -> attnT) -------
        # AV emits two concurrent col-tiled matmuls per (j, hp): V into
        # PSUM rows 0:64, ones into rows 64:128 (the softmax denominator,
        # broadcast across 64 partitions for free).
        attnT = persist.tile([P, KD, TOK], BF16, tag="attnT")

        def av_pair(av, hp, lhs_v, probs_ap, start, stop):
            # V into rows 0:64 (col grp 0-1); the ones/denominator matmul
            # into rows 64:128 runs concurrently (col grp 2-3).
            nc.tensor.matmul(av[0:64, hp, :], lhs_v, probs_ap,
                             start=start, stop=stop, tile_position=(0, 0))
            nc.tensor.matmul(av[64:128, hp, :], ones64[:], probs_ap,
                             start=start, stop=stop, tile_position=(0, 64))

        for pr in range(NPR):
            av = av_ps.tile([P, 2, TOK], F32, tag="av", name="av")
            # local tiles from the stash
            for jl in range(TT):
                for hp in range(2):
                    h = 2 * pr + hp
                    av_pair(av, hp,
                            v_loc[:, h, jl * 64:(jl + 1) * 64],
                            probs_loc[:, pr, jl, hp, :],
                            start=(jl == 0), stop=False)
            # gathered tiles (all four slots; own slot masked to zero).
            # AV for tile j is emitted two iterations behind its scores so
            # the in-order PE never waits out the exp latency.
            pending = []

            def emit_av(jr, probs):
                ri, jj = divmod(jr, TT)
                for hp in range(2):
                    h = 2 * pr + hp
                    av_pair(av, hp,
                            v_rem[:, ri, h, jj * 64:(jj + 1) * 64],
                            probs[:, hp, :],
                            start=False, stop=(jr == GS * TT - 1))

            for jr in range(GS * TT):
                ri, jj = divmod(jr, TT)
                sp = mm_ps.tile([P, 2, TOK], F32, tag="mm2", name="mm_sc")
                for hp in range(2):
                    lo = hp * 64
                    nc.tensor.matmul(
                        sp[:, hp, :],
                        kT_rem[lo:lo + 64, pr, ri, jj * P:(jj + 1) * P],
                        qT[lo:lo + 64, pr, :], start=True, stop=True)
                probs = probsp.tile([P, 2, TOK], BF16, tag="probs",
                                    name="probs")
                if jr in DVE_EXP_JR:
                    dve_exp16(probs[:].rearrange("p a b -> p (a b)"),
                              sp[:].rearrange("p a b -> p (a b)"),
                              mdve_sb[:, jr:jr + 1])
                else:
                    nc.scalar.activation(probs[:], sp[:], Act.Exp,
                                         scale=0.125,
                                         bias=mact_sb[:, jr:jr + 1])
                pending.append((jr, probs))
                if len(pending) > 2:
                    emit_av(*pending.pop(0))
            while pending:
                emit_av(*pending.pop(0))
            # normalize. Constraints (HW-verified): custom-DVE recip only
            # works at base partition 0; tensor_copy may shift bases; TT
            # may shift its OUT base if both ins share a base. So: copy
            # the denominator rows down to base 0, one recip, two TTs.
            den = act1.tile([64, 2, TOK], F32, tag="nden", name="nden")
            rbc = act1.tile([64, 2, TOK], F32, tag="nrbc", name="nrbc")
            for hp in range(2):
                nc.vector.tensor_copy(den[:, hp, :], av[64:128, hp, :])
            nc.vector.reciprocal_approx_fast(
                rbc[:].rearrange("p a b -> p (a b)"),
                den[:].rearrange("p a b -> p (a b)"))
            nc.vector.tensor_tensor(attnT[0:64, pr, :], av[0:64, 0, :],
                                    rbc[:, 0, :], op=Alu.mult)
            nc.vector.tensor_tensor(attnT[64:128, pr, :], av[0:64, 1, :],
                                    rbc[:, 1, :], op=Alu.mult)

        # ---------------- phase 4: Wo (full PSUM accumulation) --------
        for c in range(2):
            for qth in range(2):
                ps = mm_ps.tile([P, 2, 512], F32, tag="mm2", name="mm_wo")
                for q2 in range(2):
                    qt = 2 * qth + q2
                    for pr in range(NPR):
                        nc.tensor.matmul(
                            ps[:, q2, :],
                            attnT[:, pr, qt * P:(qt + 1) * P],
                            wo_sb[:, pr, c * 512:(c + 1) * 512],
                            start=(pr == 0), stop=(pr == NPR - 1))
                for q2 in range(2):
                    qt = 2 * qth + q2
                    sl = x1_sb[:, qt, c * 512:(c + 1) * 512]
                    nc.vector.tensor_add(sl, sl, ps[:, q2, :])

        if DEBUG:
            nc.scalar.dma_start(dbg["d_attnT"][:], attnT[:])
            nc.scalar.dma_start(dbg["d_x1a"][:], x1_sb[:])

        # ---------------- phase 5: LN2 + transpose ----------------
        mT = persist.tile([P, KD, TOK], BF16, tag="actT")

        def _ln2_apply(t, mt):
            for k in range(KD):
                pe_transpose(mT[:, k, t * P:(t + 1) * P],
                             mt[:, k * P:(k + 1) * P])

        layer_norm_all(x1_sb, _ln2_apply)
        # residual picks up the W2 bias here (after LN2 consumed x1)
        for t in range(TT):
            nc.vector.tensor_tensor(x1_sb[:, t, :], x1_sb[:, t, :],
                                    b2_bc[:], op=Alu.add)

        # ---------------- phase 6: W1 + gelu ----------------
        g1T = persist.tile([P, FT, TOK], BF16, tag="wqk_g1T")
        for mp in range(FT // 2):
            if mp % 2 == 0 and 2 <= mp and mp // 2 + 2 < 8:
                # refill chunk mp//2+2: aliases chunk mp//2-1, whose
                # readers (mp-2, mp-1) are already emitted
                ch = mp // 2 + 2
                t_ = persist.tile([P, KD, 512], BF16, tag=f"ws{ch % 3}",
                                  name=f"w1c{ch}")
                nc.sync.dma_start(t_[:], w1_ext[ch])
                w1c.append(t_)
            wt = w1c[mp // 2]
            mo = mp % 2
            ps = mm_ps.tile([P, 2, TOK], F32, tag="mm2", name="mm_w1")
            for hf in range(2):
                for k in range(KD):
                    nc.tensor.matmul(ps[:, hf, :],
                                     wt[:, k, (2 * mo + hf) * P:
                                        (2 * mo + hf + 1) * P],
                                     mT[:, k, :],
                                     start=(k == 0), stop=(k == KD - 1))
            for hf in range(2):
                m = 2 * mp + hf
                nc.scalar.activation(g1T[:, m, :], ps[:, hf, :],
                                     Act.Gelu_apprx_tanh,
                                     bias=b1col[:, m:m + 1])

        if DEBUG:
            nc.scalar.dma_start(dbg["d_g1T"][:], g1T[:])

        # ---------------- phase 7: W2 (8 parallel chains) ------------
        # 8 chains (c, qt) in 4 PSUM tiles: 2 from mm_ps + 2 from av_ps.
        w2c = []
        for ch in range(3):
            t_ = persist.tile([P, 4, D], BF16, tag=f"ws{ch % 3}",
                              name=f"w2c{ch}")
            nc.sync.dma_start(t_[:], w2_ext[ch])
            w2c.append(t_)

        pss = [mm_ps.tile([P, 2, 512], F32, tag="mm2", name="mm_w2")
               for _ in range(2)]
        pss += [av_ps.tile([P, 2, 512], F32, tag="av", name="mm_w2b")
                for _ in range(2)]

        def chain(c, qt):
            t_ = pss[c * 2 + qt // 2]
            return t_[:, qt % 2, :]

        out_q = [nc.scalar, nc.sync, nc.gpsimd, nc.sync]

        def finish_chain(c, qt, qi):
            ot = act1.tile([P, 512], F32, tag=f"oout{qt % 2}", name="oout")
            nc.vector.scalar_tensor_tensor(
                ot[:], chain(c, qt), 1.0,
                x1_sb[:, qt, c * 512:(c + 1) * 512],
                op0=Alu.mult, op1=Alu.add)
            out_q[qi % 4].dma_start(
                out_ext[qt * P:(qt + 1) * P, c * 512:(c + 1) * 512],
                ot[:])

        for ch in range(8):
            if 1 <= ch and ch + 2 < 8:
                # refill chunk ch+2: aliases ch-1, whose readers are emitted
                t_ = persist.tile([P, 4, D], BF16, tag=f"ws{(ch + 2) % 3}",
                                  name=f"w2c{ch + 2}")
                nc.sync.dma_start(t_[:], w2_ext[ch + 2])
                w2c.append(t_)
            if ch < 7:
                for fl in range(4):
                    ff = ch * 4 + fl
                    for c in range(2):
                        for qt in range(TT):
                            nc.tensor.matmul(
                                chain(c, qt),
                                g1T[:, ff, qt * P:(qt + 1) * P],
                                w2c[ch][:, fl, c * 512:(c + 1) * 512],
                                start=(ff == 0), stop=False)
            else:
                # last chunk: finish chain-by-chain so the evacuations
                # pipeline with the remaining matmuls instead of
                # serializing after the final one
                qi = 0
                for c in range(2):
                    for qt in range(TT):
                        for fl in range(4):
                            ff = ch * 4 + fl
                            nc.tensor.matmul(
                                chain(c, qt),
                                g1T[:, ff, qt * P:(qt + 1) * P],
                                w2c[ch][:, fl, c * 512:(c + 1) * 512],
                                start=False, stop=(fl == 3))
                        finish_chain(c, qt, qi)
                        qi += 1

    nc.compile()
    return nc


def _get_nc():
    if "nc" not in _cache:
        _cache["nc"] = _build()
    return _cache["nc"]


def kernel(**inputs):
    from concourse.bass_utils import run_bass_kernel_spmd

    nc = _get_nc()
    in_maps = prepare_in_maps(inputs)
    res = run_bass_kernel_spmd(nc, in_maps, core_ids=list(range(NCORES)))
    out = np.concatenate([res.results[c]["out"] for c in range(NCORES)],
                         axis=0)
    return out.reshape(B, S, D).astype(np.float32)


# revision 18
# speedup vs baseline: 1.5325x; 1.1351x over previous
"""Distributed Trainium2 kernel for a dense transformer block (v5).

Sharding: sequence-parallel over the 8 NeuronCores. The flattened
[B*S=4096, D=1024] token stream is split into 8 contiguous shards of 512
tokens (cores 0-3 hold batch 0, cores 4-7 hold batch 1). Weights are
replicated. Collectives: an AllGather of K^T right after the K GEMM and
a second AllGather of V right after the V GEMM, both within the 4-core
batch group, so the rings overlap Q / local-attention compute.

v5 highlights (trace-driven):
 - Softmax denominators come from a col-tiled ones-matmul into PSUM
   partitions 64..127 that runs CONCURRENTLY with the V matmul
   (cols 0..63) - measured 0ns for the second matmul of each pair.
   Normalization is then reciprocal_approx_fast([64,512]) + one
   tensor_tensor, killing the old dens-copy/broadcast/reciprocal chain.
 - Gathered-pass probs are bf16: ScalarE exp writes bf16; the VectorE
   share uses a ONE-op Schraudolph (int16 output = top 16 bits of the
   f32 trick, bitcast to bf16). AV matmuls mix fp8 V x bf16 probs
   (validated exact on HW).
 - V is stored 64-wide/contiguous (no interleaved ones column), so the
   bounce is one DMA and the remote unpack runs at 256B granularity.
 - Split collectives (K, then V), batched unpacks on the Sync queue,
   no bias matmuls, W2 epilogue staggered per chain.
"""

import sys

if "/opt/trn_rl_repo" not in sys.path:
    sys.path.insert(0, "/opt/trn_rl_repo")

import numpy as np

B, S, D = 2, 2048, 1024
H, DH, FF = 16, 64, 4096
NCORES = 8
TOK = (B * S) // NCORES      # 512 tokens per core
P = 128
TT = TOK // P                # 4 token tiles
KD = D // P                  # 8 contract tiles over D
FT = FF // P                 # 32 tiles over FF
GS = 4                       # group size (cores per batch)
NKJ = S // P                 # 16 key tiles per batch
NPR = H // 2                 # 8 head pairs
GROUPS = [[0, 1, 2, 3], [4, 5, 6, 7]]
KELEMS = KD * P * TOK        # fp8 elements per bounce region (K or V)

# Schraudolph exp: exp(x) ~= bitcast_f32(int32(x*A + B)); A folds the
# 1/sqrt(DH) score scale. The /65536 variants produce the TOP 16 bits
# directly as an int16, which bitcast as bf16.
EXP_AF = 12102203.161561485
EXP_A = EXP_AF * 0.125
EXP_B = 1064986823.0
EXP_A16 = EXP_A / 65536.0
EXP_B16 = EXP_B / 65536.0
DVE_EXP_JL = frozenset((1, 3))             # local-pass j tiles on VectorE
DVE_EXP_JR = frozenset((2, 5, 8, 11, 14))  # gathered-pass j tiles on VectorE

_cache = {}
DEBUG = False


def _prep(inputs):
    """Host-side: fold LN affines + V bias into weights, cast/arrange."""
    import ml_dtypes

    BF = ml_dtypes.bfloat16
    f32 = {k: np.asarray(v, dtype=np.float32) for k, v in inputs.items()}

    wqkv = f32["Wqkv"] * f32["ln1_g"][:, None]
    bqkv = f32["bqkv"] + f32["ln1_b"] @ f32["Wqkv"]
    w1 = f32["W1"] * f32["ln2_g"][:, None]
    b1 = f32["b1"] + f32["ln2_b"] @ f32["W1"]
    # softmax rows sum to 1, so attn(v + bv) = attn(v) + bv; fold the V
    # bias through Wo into the Wo bias.
    bo_eff = f32["bo"] + bqkv[2 * D:] @ f32["Wo"]

    def colmajor(w, nk):
        # [nk*P, M] -> [P, nk, M]
        return np.ascontiguousarray(
            w.reshape(nk, P, w.shape[1]).transpose(1, 0, 2))

    w1cm = colmajor(w1, KD)                       # [P, KD, FF]
    w1ch = np.ascontiguousarray(                  # [8, P, KD, 512]
        w1cm.reshape(P, KD, 8, 512).transpose(2, 0, 1, 3))  # -> BF below
    w2cm = colmajor(f32["W2"], FT)                # [P, FT, D]
    w2ch = np.ascontiguousarray(                  # [8, P, 4, D]
        w2cm.reshape(P, 8, 4, D).transpose(1, 0, 2, 3))

    F8 = ml_dtypes.float8_e4m3
    WS = 16.0   # fp8 weight scale; scale-down folded into the evacuations

    def f8w(a):
        return np.clip(a * WS, -240, 240).astype(F8)

    wk = {
        "wqk": f8w(colmajor(wqkv[:, :2 * D], KD)),
        "wv": f8w(colmajor(wqkv[:, 2 * D:], KD)),
        "wo": f8w(colmajor(f32["Wo"], KD)),
        "w1ch": w1ch.astype(BF),
        "w2ch": f8w(w2ch),
        # qk bias as a per-partition column per m-tile: [P, 16]
        "bqkcol2": np.ascontiguousarray(
            bqkv[:2 * D].reshape(2 * KD, P).transpose(1, 0)).astype(
                np.float32),
        "borow": np.ascontiguousarray(bo_eff[None, :]).astype(BF),
        "b2row": np.ascontiguousarray(f32["b2"][None, :]).astype(BF),
        "b1col": np.ascontiguousarray(
            b1.reshape(FT, P).transpose(1, 0)).astype(np.float32),
    }
    x = np.ascontiguousarray(f32["x"]).reshape(B * S, D).astype(BF)
    return x, wk


def prepare_in_maps(inputs):
    x, wk = _prep(inputs)
    in_maps = []
    for c in range(NCORES):
        rank = c % GS
        # exp-bias masks: kill the own-rank key tiles in the gathered pass
        # (their true contribution comes from the local stash instead)
        mask_act = np.zeros((P, NKJ), np.float32)
        mask_dve = np.full((P, NKJ), EXP_B16, np.float32)
        mask_act[:, rank * TT:(rank + 1) * TT] = -80.0
        mask_dve[:, rank * TT:(rank + 1) * TT] = \
            (EXP_B - 80.0 * EXP_AF) / 65536.0
        m = {"x": np.ascontiguousarray(x[c * TOK:(c + 1) * TOK]),
             "mask_act": mask_act, "mask_dve": mask_dve}
        m.update(wk)
        in_maps.append(m)
    return in_maps


def _build():
    from contextlib import ExitStack
    from concourse import bacc, bass, tile, mybir
    from concourse.masks import make_identity

    F32 = mybir.dt.float32
    BF16 = mybir.dt.bfloat16
    F8 = mybir.dt.float8e4
    I16 = mybir.dt.int16
    I32 = mybir.dt.int32
    Alu = mybir.AluOpType
    Act = mybir.ActivationFunctionType

    nc = bacc.Bacc("TRN2", target_bir_lowering=False, debug=False,
                   num_devices=NCORES)

    x_ext = nc.dram_tensor("x", [TOK, D], BF16, kind="ExternalInput")
    wqk_ext = nc.dram_tensor("wqk", [P, KD, 2 * D], F8, kind="ExternalInput")
    wv_ext = nc.dram_tensor("wv", [P, KD, D], F8, kind="ExternalInput")
    wo_ext = nc.dram_tensor("wo", [P, KD, D], F8, kind="ExternalInput")
    w1_ext = nc.dram_tensor("w1ch", [8, P, KD, 512], BF16,
                            kind="ExternalInput")
    w2_ext = nc.dram_tensor("w2ch", [8, P, 4, D], F8, kind="ExternalInput")
    bqkcol2_ext = nc.dram_tensor("bqkcol2", [P, 2 * KD], F32,
                                 kind="ExternalInput")
    borow_ext = nc.dram_tensor("borow", [1, D], BF16, kind="ExternalInput")
    b2row_ext = nc.dram_tensor("b2row", [1, D], BF16, kind="ExternalInput")
    b1col_ext = nc.dram_tensor("b1col", [P, FT], F32, kind="ExternalInput")
    mact_ext = nc.dram_tensor("mask_act", [P, NKJ], F32, kind="ExternalInput")
    mdve_ext = nc.dram_tensor("mask_dve", [P, NKJ], F32, kind="ExternalInput")
    out_ext = nc.dram_tensor("out", [TOK, D], F32, kind="ExternalOutput")
    dbg = {}
    if DEBUG:
        for nm, shp, dt in [
                ("d_hT", [P, KD, TOK], F8), ("d_kTloc", [P, KD, TOK], F8),
                ("d_qT", [P, KD, TOK], F8), ("d_vloc", [P, H, TT * 64], F8),
                ("d_ktrem", [P, KD, GS, TOK], F8),
                ("d_vrem", [P, GS, H, TT * 64], F8),
                ("d_ploc", [P, NPR, TT, 2, TOK], F8),
                ("d_attnT", [P, KD, TOK], F8),
                ("d_x1a", [P, TT, D], BF16), ("d_g1T", [P, FT, TOK], F8)]:
            dbg[nm] = nc.dram_tensor(nm, shp, dt, kind="ExternalOutput")

    with tile.TileContext(nc) as tc, ExitStack() as ctx:
        const = ctx.enter_context(tc.tile_pool(name="const", bufs=1))
        persist = ctx.enter_context(tc.tile_pool(name="persist", bufs=1))
        act = ctx.enter_context(tc.tile_pool(name="act", bufs=2))
        act1 = ctx.enter_context(tc.tile_pool(name="act1", bufs=1))
        probsp = ctx.enter_context(tc.tile_pool(name="probsp", bufs=3))
        mm_ps = ctx.enter_context(
            tc.tile_pool(name="mm_ps", bufs=2, space="PSUM"))
        av_ps = ctx.enter_context(
            tc.tile_pool(name="av_ps", bufs=2, space="PSUM"))
        dram = ctx.enter_context(tc.tile_pool(name="dram", bufs=1,
                                              space="DRAM"))

        # ---------------- input DMAs ----------------
        # x tile-by-tile so LN1 stats start after ~1/4 of the load
        x1_sb = persist.tile([P, TT, D], BF16, tag="x1")
        for th in range(TT):
            nc.sync.dma_start(
                x1_sb[:, th:th + 1, :],
                x_ext[th * P:(th + 1) * P, :].rearrange(
                    "(t p) d -> p t d", p=P))
        wqk_sb = persist.tile([P, KD, 2 * D], F8, tag="wqk_g1T")
        # K-half of Wqk first so the K GEMM (and thus the K AllGather)
        # starts as early as possible; Q-half on another queue.
        nc.scalar.dma_start(wqk_sb[:, :, D:2 * D], wqk_ext[:, :, D:2 * D])
        nc.gpsimd.dma_start(wqk_sb[:, :, 0:D], wqk_ext[:, :, 0:D])
        wv_sb = persist.tile([P, KD, D], F8, tag="wv_wo")
        nc.sync.dma_start(wv_sb[:], wv_ext[:])

        # ---------------- constants ----------------
        eps_t = const.tile([P, 1], F32)
        nc.vector.memset(eps_t[:], 1e-5)
        ones64 = const.tile([P, 64], F8)
        nc.vector.memset(ones64[:], 1.0)
        ident = const.tile([P, P], BF16)
        make_identity(nc, ident[:])
        bqkcol2 = const.tile([P, 2 * KD], F32)
        nc.scalar.dma_start(bqkcol2[:], bqkcol2_ext[:])
        borow = const.tile([1, D], BF16)
        nc.scalar.dma_start(borow[:], borow_ext[:])
        b2row = const.tile([1, D], BF16)
        nc.scalar.dma_start(b2row[:], b2row_ext[:])
        b1col = const.tile([P, FT], F32)
        nc.scalar.dma_start(b1col[:], b1col_ext[:])
        mact_sb = const.tile([P, NKJ], F32)
        nc.sync.dma_start(mact_sb[:], mact_ext[:])
        mdve_sb = const.tile([P, NKJ], F32)
        nc.sync.dma_start(mdve_sb[:], mdve_ext[:])
        # broadcast bias rows for the residual adds (gpsimd is idle now)
        bo_bc = const.tile([P, D], BF16)
        nc.gpsimd.partition_broadcast(bo_bc[:], borow[:])
        b2_bc = const.tile([P, D], BF16)
        nc.gpsimd.partition_broadcast(b2_bc[:], b2row[:])

        # ---------------- helpers ----------------
        def layer_norm_all(src_tile, apply_fn):
            # fully per-tile so tile 0's transposes start immediately
            for t in range(TT):
                stats = act.tile([P, 2, 6], F32, tag="ln_stats",
                                 name="ln_stats")
                nc.vector.bn_stats(stats[:, 0, :], src_tile[:, t, 0:512])
                nc.vector.bn_stats(stats[:, 1, :], src_tile[:, t, 512:1024])
                mv = act.tile([P, 2], F32, tag="ln_mv", name="ln_mv")
                nc.vector.bn_aggr(mv[:], stats[:])
                rs = act.tile([P, 1], F32, tag="ln_rs", name="ln_rs")
                nc.scalar.activation(rs[:], mv[:, 1:2], Act.Sqrt,
                                     bias=eps_t[:])
                nc.vector.reciprocal(rs[:], rs[:])
                ht = act.tile([P, D], BF16, tag="hmt", name="hmt")
                nc.vector.tensor_scalar(ht[:], src_tile[:, t, :],
                                        scalar1=mv[:, 0:1],
                                        scalar2=rs[:, 0:1],
                                        op0=Alu.subtract, op1=Alu.mult)
                apply_fn(t, ht)

        def pe_transpose(dst_ap, src_ap):
            tp = mm_ps.tile([P, P], BF16, tag="mm2", name="tp_ps")
            nc.tensor.transpose(tp[:], src_ap, ident[:])
            nc.vector.tensor_copy(dst_ap, tp[:])

        def dve_exp16(probs_bf16_flat, sp_ap_flat, bconst):
            # one-op Schraudolph: int16(x*A/65536 + B/65536) == top 16
            # bits of the f32 trick == the bf16 exp approximation
            nc.vector.tensor_scalar(probs_bf16_flat.bitcast(I16), sp_ap_flat,
                                    scalar1=EXP_A16, scalar2=bconst,
                                    op0=Alu.mult, op1=Alu.add)

        def dve_exp8(probs_f8_flat, sp_ap_flat):
            # two-op Schraudolph for the fp8 local stash
            ei = act1.tile([P, 2 * TOK], I32, tag="expi", name="expi")
            nc.vector.tensor_scalar(ei[:], sp_ap_flat,
                                    scalar1=EXP_A, scalar2=EXP_B,
                                    op0=Alu.mult, op1=Alu.add)
            nc.vector.tensor_copy(probs_f8_flat, ei[:].bitcast(F32))

        # ---------------- phase 1: LN1 + transpose ----------------
        hT = persist.tile([P, KD, TOK], F8, tag="actT")

        def _ln1_apply(t, ht):
            for k in range(KD):
                pe_transpose(hT[:, k, t * P:(t + 1) * P],
                             ht[:, k * P:(k + 1) * P])

        layer_norm_all(x1_sb, _ln1_apply)

        # ---------------- phase 2: K, CC-K, V, CC-V, Q ----------------
        qT = persist.tile([P, KD, TOK], F8, tag="qT")
        kT_loc = persist.tile([P, KD, TOK], F8, tag="kTloc")

        DR = mybir.MatmulPerfMode.DoubleRow

        def qk_block(mp, is_k):
            ps = mm_ps.tile([P, 2, TOK], F32, tag="mm2", name="mm_qkv")
            for hf in range(2):
                m = 2 * mp + hf
                for kk in range(KD // 2):
                    nc.tensor.matmul(ps[:, hf, :],
                                     wqk_sb[:, 2 * kk:2 * kk + 2,
                                            m * P:(m + 1) * P],
                                     hT[:, 2 * kk:2 * kk + 2, :],
                                     start=(kk == 0), stop=(kk == KD // 2 - 1),
                                     perf_mode=DR)
            for hf in range(2):
                m = 2 * mp + hf
                dst = kT_loc[:, m - 8, :] if is_k else qT[:, m, :]
                nc.scalar.activation(dst, ps[:, hf, :], Act.Identity,
                                     scale=1.0 / 16.0,
                                     bias=bqkcol2[:, m:m + 1])

        for mp in range(4, 8):          # K first
            qk_block(mp, is_k=True)

        # K bounce + AllGather (starts while V/Q still compute)
        cc_in_k = dram.tile([KELEMS], F8)
        cc_out_k = dram.tile([GS * KELEMS], F8)
        nc.gpsimd.dma_start(
            cc_in_k[:].rearrange("(k p t) -> p k t", k=KD, p=P),
            kT_loc[:])
        nc.gpsimd.collective_compute(
            "AllGather", Alu.bypass, ins=[cc_in_k[:]], outs=[cc_out_k[:]],
            replica_groups=GROUPS)

        # V: pure 64-wide layout [P, H, TT*64]; wire format (p, h, t*f)
        v_loc = persist.tile([P, H, TT * 64], F8, tag="vloc")
        v_rem = persist.tile([P, GS, H, TT * 64], F8, tag="vrem")
        for c in range(2):
            pss = [mm_ps.tile([P, 2, 512], F32, tag="mm2", name="mm_v")
                   for _ in range(2)]
            for kk in range(KD // 2):
                for t in range(TT):
                    nc.tensor.matmul(pss[t // 2][:, t % 2, :],
                                     hT[:, 2 * kk:2 * kk + 2,
                                        t * P:(t + 1) * P],
                                     wv_sb[:, 2 * kk:2 * kk + 2,
                                           c * 512:(c + 1) * 512],
                                     start=(kk == 0),
                                     stop=(kk == KD // 2 - 1), perf_mode=DR)
            for t in range(TT):
                nc.vector.tensor_scalar_mul(
                    v_loc[:, c * 8:(c + 1) * 8, t * 64:(t + 1) * 64],
                    pss[t // 2][:, t % 2, :].rearrange(
                        "p (h f) -> p h f", h=8), 1.0 / 16.0)

        # V bounce + AllGather (one contiguous DMA)
        cc_in_v = dram.tile([KELEMS], F8)
        cc_out_v = dram.tile([GS * KELEMS], F8)
        nc.gpsimd.dma_start(
            cc_in_v[:].rearrange("(p h f) -> p h f", p=P, h=H),
            v_loc[:])
        nc.gpsimd.collective_compute(
            "AllGather", Alu.bypass, ins=[cc_in_v[:]], outs=[cc_out_v[:]],
            replica_groups=GROUPS)

        # Q overlaps the rings
        for mp in range(0, 4):
            qk_block(mp, is_k=False)

        # weight prefetch during the rings
        wo_sb = persist.tile([P, KD, D], F8, tag="wv_wo")
        nc.scalar.dma_start(wo_sb[:], wo_ext[:])

        # batched remote unpack on the (idle) Sync queue; these wait on
        # the collectives' completion sems without blocking compute
        kT_rem = persist.tile([P, KD, GS, TOK], F8, tag="ktrem")
        for r in range(GS):
            nc.sync.dma_start(
                kT_rem[:, :, r, :],
                cc_out_k[r * KELEMS:(r + 1) * KELEMS].rearrange(
                    "(k p t) -> p k t", k=KD, p=P))
        for r in range(GS):
            nc.sync.dma_start(
                v_rem[:, r, :, :],
                cc_out_v[r * KELEMS:(r + 1) * KELEMS].rearrange(
                    "(p h f) -> p h f", p=P, h=H))

        # residual picks up the (folded) Wo bias during the ring window
        for t in range(TT):
            nc.vector.tensor_tensor(x1_sb[:, t, :], x1_sb[:, t, :],
                                    bo_bc[:], op=Alu.add)

        # local attention pass during the rings: probs for own 4 key tiles
        probs_loc = persist.tile([P, NPR, TT, 2, TOK], F8, tag="ploc")
        for pr in range(NPR):
            for jl in range(TT):
                sp = mm_ps.tile([P, 2, TOK], F32, tag="mm2", name="mm_scl")
                for hp in range(2):
                    lo = hp * 64
                    nc.tensor.matmul(
                        sp[:, hp, :],
                        kT_loc[lo:lo + 64, pr, jl * P:(jl + 1) * P],
                        qT[lo:lo + 64, pr, :], start=True, stop=True)
                pl = probs_loc[:, pr, jl, :, :]
                if jl in DVE_EXP_JL:
                    dve_exp8(pl.rearrange("p a b -> p (a b)"),
                             sp[:].rearrange("p a b -> p (a b)"))
                else:
                    nc.scalar.activation(pl, sp[:], Act.Exp, scale=0.125)

        if DEBUG:
            nc.scalar.dma_start(dbg["d_hT"][:], hT[:])
            nc.scalar.dma_start(dbg["d_kTloc"][:], kT_loc[:])
            nc.scalar.dma_start(dbg["d_qT"][:], qT[:])
            nc.scalar.dma_start(dbg["d_vloc"][:], v_loc[:])
            nc.scalar.dma_start(dbg["d_ktrem"][:], kT_rem[:])
            nc.scalar.dma_start(dbg["d_vrem"][:], v_rem[:])
            nc.scalar.dma_start(dbg["d_ploc"][:], probs_loc[:])

        # W1 stream chunks: manual double-buffer via tags. Only the first
        # three are prefetched here; the rest are emitted just-in-time
        # inside the W1 loop AFTER their buffer's previous readers, so the
        # WAR dependency is correctly formed.
        w1c = []
        for ch in range(3):
            t_ = persist.tile([P, KD, 512], BF16, tag=f"ws{ch % 3}",
                              name=f"w1c{ch}")
            nc.sync.dma_start(t_[:], w1_ext[ch])
            w1c.append(t_)

        # ------- phase 3: attention (V-stationary AV -> attnT) -------
        # AV emits two concurrent col-tiled matmuls per (j, hp): V into
        # PSUM rows 0:64, ones into rows 64:128 (the softmax denominator,
        # broadcast across 64 partitions for free).
        attnT = persist.tile([P, KD, TOK], F8, tag="attnT")

        def av_pair(av, hp, lhs_v, probs_ap, start, stop):
            # V into rows 0:64 (col grp 0-1); the ones/denominator matmul
            # into rows 64:128 runs concurrently (col grp 2-3).
            nc.tensor.matmul(av[0:64, hp, :], lhs_v, probs_ap,
                             start=start, stop=stop, tile_position=(0, 0))
            nc.tensor.matmul(av[64:128, hp, :], ones64[:], probs_ap,
                             start=start, stop=stop, tile_position=(0, 64))

        for pr in range(NPR):
            av = av_ps.tile([P, 2, TOK], F32, tag="av", name="av")
            # local tiles from the stash
            for jl in range(TT):
                for hp in range(2):
                    h = 2 * pr + hp
                    av_pair(av, hp,
                            v_loc[:, h, jl * 64:(jl + 1) * 64],
                            probs_loc[:, pr, jl, hp, :],
                            start=(jl == 0), stop=False)
            # gathered tiles (all four slots; own slot masked to zero).
            # AV for tile j is emitted two iterations behind its scores so
            # the in-order PE never waits out the exp latency.
            pending = []

            def emit_av(jr, probs):
                ri, jj = divmod(jr, TT)
                for hp in range(2):
                    h = 2 * pr + hp
                    av_pair(av, hp,
                            v_rem[:, ri, h, jj * 64:(jj + 1) * 64],
                            probs[:, hp, :],
                            start=False, stop=(jr == GS * TT - 1))

            for jr in range(GS * TT):
                ri, jj = divmod(jr, TT)
                sp = mm_ps.tile([P, 2, TOK], F32, tag="mm2", name="mm_sc")
                for hp in range(2):
                    lo = hp * 64
                    nc.tensor.matmul(
                        sp[:, hp, :],
                        kT_rem[lo:lo + 64, pr, ri, jj * P:(jj + 1) * P],
                        qT[lo:lo + 64, pr, :], start=True, stop=True)
                probs = probsp.tile([P, 2, TOK], BF16, tag="probs",
                                    name="probs")
                if jr in DVE_EXP_JR:
                    dve_exp16(probs[:].rearrange("p a b -> p (a b)"),
                              sp[:].rearrange("p a b -> p (a b)"),
                              mdve_sb[:, jr:jr + 1])
                else:
                    nc.scalar.activation(probs[:], sp[:], Act.Exp,
                                         scale=0.125,
                                         bias=mact_sb[:, jr:jr + 1])
                pending.append((jr, probs))
                if len(pending) > 2:
                    emit_av(*pending.pop(0))
            while pending:
                emit_av(*pending.pop(0))
            # normalize. Constraints (HW-verified): custom-DVE recip only
            # works at base partition 0; tensor_copy may shift bases; TT
            # may shift its OUT base if both ins share a base. So: copy
            # the denominator rows down to base 0, one recip, two TTs.
            den = act1.tile([64, 2, TOK], F32, tag="nden", name="nden")
            rbc = act1.tile([64, 2, TOK], F32, tag="nrbc", name="nrbc")
            for hp in range(2):
                # rbc = 16/den so attnT is stored x16 (fp8-friendly range)
                nc.vector.tensor_scalar_mul(den[:, hp, :],
                                             av[64:128, hp, :], 1.0 / 16.0)
            nc.vector.reciprocal_approx_fast(
                rbc[:].rearrange("p a b -> p (a b)"),
                den[:].rearrange("p a b -> p (a b)"))
            nc.vector.tensor_tensor(attnT[0:64, pr, :], av[0:64, 0, :],
                                    rbc[:, 0, :], op=Alu.mult)
            nc.vector.tensor_tensor(attnT[64:128, pr, :], av[0:64, 1, :],
                                    rbc[:, 1, :], op=Alu.mult)

        # ---------------- phase 4: Wo (full PSUM accumulation) --------
        for c in range(2):
            for qth in range(2):
                ps = mm_ps.tile([P, 2, 512], F32, tag="mm2", name="mm_wo")
                for q2 in range(2):
                    qt = 2 * qth + q2
                    for pk in range(NPR // 2):
                        nc.tensor.matmul(
                            ps[:, q2, :],
                            attnT[:, 2 * pk:2 * pk + 2,
                                  qt * P:(qt + 1) * P],
                            wo_sb[:, 2 * pk:2 * pk + 2,
                                  c * 512:(c + 1) * 512],
                            start=(pk == 0), stop=(pk == NPR // 2 - 1),
                            perf_mode=DR)
                for q2 in range(2):
                    qt = 2 * qth + q2
                    sl = x1_sb[:, qt, c * 512:(c + 1) * 512]
                    # attnT is x16 and wo is x16 -> scale by 1/256
                    nc.vector.scalar_tensor_tensor(
                        sl, ps[:, q2, :], 1.0 / 256.0, sl,
                        op0=Alu.mult, op1=Alu.add)

        if DEBUG:
            nc.scalar.dma_start(dbg["d_attnT"][:], attnT[:])
            nc.scalar.dma_start(dbg["d_x1a"][:], x1_sb[:])

        # ---------------- phase 5: LN2 + transpose ----------------
        mT = persist.tile([P, KD, TOK], BF16, tag="actT2")

        def _ln2_apply(t, mt):
            for k in range(KD):
                pe_transpose(mT[:, k, t * P:(t + 1) * P],
                             mt[:, k * P:(k + 1) * P])

        layer_norm_all(x1_sb, _ln2_apply)
        # residual picks up the W2 bias here (after LN2 consumed x1)
        for t in range(TT):
            nc.vector.tensor_tensor(x1_sb[:, t, :], x1_sb[:, t, :],
                                    b2_bc[:], op=Alu.add)

        # ---------------- phase 6: W1 + gelu ----------------
        g1T = persist.tile([P, FT, TOK], F8, tag="wqk_g1T")
        for mp in range(FT // 2):
            if mp % 2 == 0 and 2 <= mp and mp // 2 + 2 < 8:
                # refill chunk mp//2+2: aliases chunk mp//2-1, whose
                # readers (mp-2, mp-1) are already emitted
                ch = mp // 2 + 2
                t_ = persist.tile([P, KD, 512], BF16, tag=f"ws{ch % 3}",
                                  name=f"w1c{ch}")
                nc.sync.dma_start(t_[:], w1_ext[ch])
                w1c.append(t_)
            wt = w1c[mp // 2]
            mo = mp % 2
            ps = mm_ps.tile([P, 2, TOK], F32, tag="mm2", name="mm_w1")
            for hf in range(2):
                for k in range(KD):
                    nc.tensor.matmul(ps[:, hf, :],
                                     wt[:, k, (2 * mo + hf) * P:
                                        (2 * mo + hf + 1) * P],
                                     mT[:, k, :],
                                     start=(k == 0), stop=(k == KD - 1))
            for hf in range(2):
                m = 2 * mp + hf
                nc.scalar.activation(g1T[:, m, :], ps[:, hf, :],
                                     Act.Gelu_apprx_tanh,
                                     bias=b1col[:, m:m + 1])

        if DEBUG:
            nc.scalar.dma_start(dbg["d_g1T"][:], g1T[:])

        # ---------------- phase 7: W2 (8 parallel chains) ------------
        # 8 chains (c, qt) in 4 PSUM tiles: 2 from mm_ps + 2 from av_ps.
        w2c = []
        for ch in range(3):
            t_ = persist.tile([P, 4, D], F8, tag=f"ws{ch % 3}",
                              name=f"w2c{ch}")
            nc.sync.dma_start(t_[:], w2_ext[ch])
            w2c.append(t_)

        pss = [mm_ps.tile([P, 2, 512], F32, tag="mm2", name="mm_w2")
               for _ in range(2)]
        pss += [av_ps.tile([P, 2, 512], F32, tag="av", name="mm_w2b")
                for _ in range(2)]

        def chain(c, qt):
            t_ = pss[c * 2 + qt // 2]
            return t_[:, qt % 2, :]

        out_q = [nc.scalar, nc.sync, nc.gpsimd, nc.sync]

        def finish_chain(c, qt, qi):
            ot = act1.tile([P, 512], F32, tag=f"oout{qt % 2}", name="oout")
            nc.vector.scalar_tensor_tensor(
                ot[:], chain(c, qt), 1.0 / 16.0,
                x1_sb[:, qt, c * 512:(c + 1) * 512],
                op0=Alu.mult, op1=Alu.add)
            out_q[qi % 4].dma_start(
                out_ext[qt * P:(qt + 1) * P, c * 512:(c + 1) * 512],
                ot[:])

        for ch in range(8):
            if 1 <= ch and ch + 2 < 8:
                # refill chunk ch+2: aliases ch-1, whose readers are emitted
                t_ = persist.tile([P, 4, D], F8, tag=f"ws{(ch + 2) % 3}",
                                  name=f"w2c{ch + 2}")
                nc.sync.dma_start(t_[:], w2_ext[ch + 2])
                w2c.append(t_)
            if ch < 7:
                for fp in range(2):
                    ff = ch * 4 + 2 * fp
                    for c in range(2):
                        for qt in range(TT):
                            nc.tensor.matmul(
                                chain(c, qt),
                                g1T[:, ff:ff + 2, qt * P:(qt + 1) * P],
                                w2c[ch][:, 2 * fp:2 * fp + 2,
                                        c * 512:(c + 1) * 512],
                                start=(ff == 0), stop=False, perf_mode=DR)
            else:
                # last chunk: finish chain-by-chain, ordered so
                # consecutive finishes live in different PSUM tiles and
                # the evacuations pipeline with the remaining matmuls
                qi = 0
                for c, qt in [(0, 0), (0, 2), (1, 0), (1, 2),
                              (0, 1), (0, 3), (1, 1), (1, 3)]:
                    for fp in range(2):
                        ff = ch * 4 + 2 * fp
                        nc.tensor.matmul(
                            chain(c, qt),
                            g1T[:, ff:ff + 2, qt * P:(qt + 1) * P],
                            w2c[ch][:, 2 * fp:2 * fp + 2,
                                    c * 512:(c + 1) * 512],
                            start=False, stop=(fp == 1), perf_mode=DR)
                    finish_chain(c, qt, qi)
                    qi += 1

    nc.compile()
    return nc


def _get_nc():
    if "nc" not in _cache:
        _cache["nc"] = _build()
    return _cache["nc"]


def kernel(**inputs):
    from concourse.bass_utils import run_bass_kernel_spmd

    nc = _get_nc()
    in_maps = prepare_in_maps(inputs)
    res = run_bass_kernel_spmd(nc, in_maps, core_ids=list(range(NCORES)))
    out = np.concatenate([res.results[c]["out"] for c in range(NCORES)],
                         axis=0)
    return out.reshape(B, S, D).astype(np.float32)
